# revision 1
# baseline (speedup 1.0000x reference)
"""GCE-GNN session-graph recommendation kernel for 8 Trainium2 NeuronCores.

Strategy (per sharding hint):
  - Data-parallel over sessions for the per-session graph compute: core c
    handles sessions [16c, 16c+16), in 8 tiles of 96 rows (2 sessions).
  - Embedding gathers (emb[x], emb[nbr_ids]) done on-device per core via
    indirect DMA against the full table (each core only gathers rows for
    its own sessions).
  - AllGather of the [16,100] per-core session vectors -> [128,100].
  - Vocab-sharded final scores matmul: core c computes S @ emb[c*62500 :
    (c+1)*62500].T from a host-pretransposed embT shard, writing a
    [128, 62500] output slice; the host concatenates along vocab.
"""

import numpy as np

import concourse.bass as bass
import concourse.mybir as mybir
import concourse.tile as tile
import concourse.tile as tile_mod
import concourse.bass_utils as bass_utils
from concourse.masks import make_identity

# ---------------------------------------------------------------- constants
B, L, K, V, D = 128, 48, 12, 500000, 100
N = B * L               # 6144 nodes
C = 8                   # cores
S = B // C              # 16 sessions per core
RPC = S * L             # 768 rows per core
T = 8                   # tiles per core
P = RPC // T            # 96 rows per tile (2 sessions)
VS = V // C             # 62500 vocab rows per core
D1 = D + 1

F32 = mybir.dt.float32
I32 = mybir.dt.int32
AF = mybir.ActivationFunctionType
AX = mybir.AxisListType
ALU = mybir.AluOpType

NEG = -1e9

# ------------------------------------------------- compiler workaround patch
# This walrus build rejects >1 plain semaphore sync-wait per instruction and
# any plain wait on CTRL-type instructions (Drain).  Patch the Tile drain and
# legalize all instructions by hoisting excess waits into standalone
# InstEventSemaphore wait instructions.
_CTRL_TYPES = ("InstDrain", "InstNoOp", "InstHalt", "InstBranch")


def _patched_drain_and_barrier(self, tick_clock, wait_clock):
    nc = self.nc
    vclock = tick_clock.global_clock
    sems = wait_clock.sems.allocated()
    for proc, sem in sorted(sems.items()):
        t = vclock[proc] if proc < len(vclock) else 0
        if t > 0:
            nc.sync.wait_ge(sem, t)
    nc.sync.drain()
    nc.all_engine_barrier()
    assert self.sems is not None
    popped = nc._tile_sem_poison_stack.pop()
    assert popped is self._sem_poison
    nc.clear_and_free_semaphores(list(self.sems.allocated().values()))
    nc.all_engine_barrier()


tile_mod.TileContext._drain_and_barrier = _patched_drain_and_barrier

_legalize_counter = [0]


def legalize_waits(nc):
    n_hoisted = 0
    for fn in nc.m.functions:
        for bb in fn.blocks:
            insts = bb.instructions
            i = 0
            while i < len(insts):
                ins = insts[i]
                si = ins.sync_info
                if si is None:
                    i += 1
                    continue
                waits = list(si.on_wait)
                plain_idx = [
                    j for j, w in enumerate(waits) if w.sync_type == "semaphore"
                ]
                keep = 0 if type(ins).__name__ in _CTRL_TYPES else 1
                if len(plain_idx) <= keep:
                    i += 1
                    continue
                hoist_idx = set(plain_idx[: len(plain_idx) - keep])
                SI = type(si)
                new_insts = []
                for j in sorted(hoist_idx):
                    w = waits[j]
                    _legalize_counter[0] += 1
                    ev = mybir.InstEventSemaphore(
                        name=f"WLEG-{_legalize_counter[0]}",
                        engine=ins.engine,
                        ins=[],
                        outs=[],
                        sync_info=SI(on_wait=[w], on_update=[]),
                    )
                    nc.register_instruction(ev, overwrite=True)
                    new_insts.append(ev)
                kept = [w for j, w in enumerate(waits) if j not in hoist_idx]
                ins.sync_info = SI(on_wait=kept, on_update=list(si.on_update))
                insts[i:i] = new_insts
                n_hoisted += len(new_insts)
                i += 1 + len(new_insts)
    return n_hoisted


# ------------------------------------------------------------- device kernel
def build_nc(repeat=1, loop=None):
    nc = bass.Bass(target_bir_lowering=False)

    # DRAM I/O (per-core shapes; values differ per core via in_maps)
    emb_d = nc.dram_tensor("emb", [V, D], F32, kind="ExternalInput")
    embT_d = nc.dram_tensor("embT", [D, VS], F32, kind="ExternalInput")
    xt_d = nc.dram_tensor("xt", [P, T], I32, kind="ExternalInput")
    nbr_d = nc.dram_tensor("nbr", [P, T * K], I32, kind="ExternalInput")
    g_d = nc.dram_tensor("g", [P, T], F32, kind="ExternalInput")
    wsT_d = nc.dram_tensor("wsT", [1, RPC * K], F32, kind="ExternalInput")
    maskb_d = nc.dram_tensor("maskb", [P, T * P], F32, kind="ExternalInput")
    pos_d = nc.dram_tensor("pos", [S, D], F32, kind="ExternalInput")
    w1d_d = nc.dram_tensor("w1d", [D, D1], F32, kind="ExternalInput")
    w1r_d = nc.dram_tensor("w1r", [1, D1], F32, kind="ExternalInput")
    w1b_d = nc.dram_tensor("w1b", [D1, 1], F32, kind="ExternalInput")
    q1_d = nc.dram_tensor("q1", [D1, 1], F32, kind="ExternalInput")
    w2a_d = nc.dram_tensor("w2a", [D, D], F32, kind="ExternalInput")
    w2b_d = nc.dram_tensor("w2b", [D, D], F32, kind="ExternalInput")
    w2bias_d = nc.dram_tensor("w2bias", [1, D], F32, kind="ExternalInput")
    w3a_d = nc.dram_tensor("w3a", [D, D], F32, kind="ExternalInput")
    w3b_d = nc.dram_tensor("w3b", [D, D], F32, kind="ExternalInput")
    w3bias_d = nc.dram_tensor("w3bias", [1, D], F32, kind="ExternalInput")
    w4_d = nc.dram_tensor("w4", [D, D], F32, kind="ExternalInput")
    w5_d = nc.dram_tensor("w5", [D, D], F32, kind="ExternalInput")
    w5bias_d = nc.dram_tensor("w5bias", [1, D], F32, kind="ExternalInput")
    q2_d = nc.dram_tensor("q2", [P, D], F32, kind="ExternalInput")
    aw_d = nc.dram_tensor("aw", [D, 1], F32, kind="ExternalInput")
    ones1_d = nc.dram_tensor("ones1", [1, P], F32, kind="ExternalInput")
    rep2_d = nc.dram_tensor("rep2", [2, P], F32, kind="ExternalInput")
    rep2T48_d = nc.dram_tensor("rep2T48", [P, 2], F32, kind="ExternalInput")
    sel_d = nc.dram_tensor("sel", [S, T * P], F32, kind="ExternalInput")
    bmask_d = nc.dram_tensor("bmask", [P, T * S], F32, kind="ExternalInput")
    id16_d = nc.dram_tensor("id16", [S, S], F32, kind="ExternalInput")
    id2_d = nc.dram_tensor("id2", [2, 2], F32, kind="ExternalInput")
    scores_d = nc.dram_tensor("scores", [B, VS], F32, kind="ExternalOutput")

    with tile.TileContext(nc) as tc:
        with tc.tile_pool(name="w", bufs=1) as wp, \
             tc.tile_pool(name="dram", bufs=1, space="DRAM") as dp, \
             tc.tile_pool(name="sb", bufs=2) as sp, \
             tc.tile_pool(name="ps", bufs=2, space="PSUM") as pp:

            # ---- persistent weights/constants in SBUF
            def wtile(dram, shape, name):
                t_ = wp.tile(shape, F32, name=name)
                nc.sync.dma_start(t_[:], dram[:])
                return t_

            w1d = wtile(w1d_d, [D, D1], "w1d")
            w1r = wtile(w1r_d, [1, D1], "w1r")
            w1b = wtile(w1b_d, [D1, 1], "w1b")
            q1 = wtile(q1_d, [D1, 1], "q1")
            w2a = wtile(w2a_d, [D, D], "w2a")
            w2b = wtile(w2b_d, [D, D], "w2b")
            w2bias = wtile(w2bias_d, [1, D], "w2bias")
            w3a = wtile(w3a_d, [D, D], "w3a")
            w3b = wtile(w3b_d, [D, D], "w3b")
            w3bias = wtile(w3bias_d, [1, D], "w3bias")
            w4 = wtile(w4_d, [D, D], "w4")
            w5 = wtile(w5_d, [D, D], "w5")
            w5bias = wtile(w5bias_d, [1, D], "w5bias")
            q2 = wtile(q2_d, [P, D], "q2")
            aw = wtile(aw_d, [D, 1], "aw")
            pos = wtile(pos_d, [S, D], "pos")
            gt = wtile(g_d, [P, T], "gt")
            wsT = wtile(wsT_d, [1, RPC * K], "wsT")
            maskb = wtile(maskb_d, [P, T * P], "maskb")

            xt = wp.tile([P, T], I32, name="xt")
            nc.sync.dma_start(xt[:], xt_d[:])
            nbr = wp.tile([P, T * K], I32, name="nbr")
            nc.sync.dma_start(nbr[:], nbr_d[:])

            id96 = wp.tile([P, P], F32, name="id96")
            make_identity(nc, id96[:])
            id128 = wp.tile([B, B], F32, name="id128")
            make_identity(nc, id128[:])
            id16 = wtile(id16_d, [S, S], "id16")
            id2 = wtile(id2_d, [2, 2], "id2")
            ones1 = wtile(ones1_d, [1, P], "ones1")
            rep2 = wtile(rep2_d, [2, P], "rep2")
            rep2T48 = wtile(rep2T48_d, [P, 2], "rep2T48")
            sel = wtile(sel_d, [S, T * P], "sel")
            bmask = wtile(bmask_d, [P, T * S], "bmask")

            def emit_body(rep, fake_coll=False):
                # collective bounce buffers (per repeat: Shared DRAM allows
                # only a single writing instruction)
                s_in = dp.tile([S, D], F32, name=f"s_in{rep}",
                               tag=f"s_in{rep}")
                s_out = dp.tile([B, D], F32, name=f"s_out{rep}",
                                tag=f"s_out{rep}",
                                addr_space="Local" if fake_coll else "Shared")
                # posw3 = pos_c @ w3b  [S, D]
                posT_ps = pp.tile([D, S], F32, tag="mm", bufs=4, space="PSUM",
                                  name="posT_ps")
                nc.tensor.transpose(posT_ps[:], pos[:], id16[:])
                posT = sp.tile([D, S], F32, tag="posT", name="posT")
                nc.vector.tensor_copy(posT[:], posT_ps[:])
                posw3_ps = pp.tile([S, D], F32, tag="mm", bufs=4, space="PSUM",
                                   name="posw3_ps")
                nc.tensor.matmul(posw3_ps[:], lhsT=posT[:], rhs=w3b[:],
                                 start=True, stop=True)
                posw3 = sp.tile([S, D], F32, tag="posw3", name="posw3")
                nc.vector.tensor_copy(posw3[:], posw3_ps[:])

                S_ps = pp.tile([S, D], F32, tag="S", bufs=1, space="PSUM",
                               name="S_ps")

                for t in range(T):
                    # ---------------- gathers
                    E = sp.tile([P, K * D], F32, tag="E", bufs=3, name="E")
                    for k in range(K):
                        nc.gpsimd.indirect_dma_start(
                            out=E[:, k * D:(k + 1) * D], out_offset=None,
                            in_=emb_d[:],
                            in_offset=bass.IndirectOffsetOnAxis(
                                ap=nbr[:, t * K + k:t * K + k + 1], axis=0))
                    hid = sp.tile([P, D], F32, tag="hid", bufs=3, name="hid")
                    nc.gpsimd.indirect_dma_start(
                        out=hid[:], out_offset=None, in_=emb_d[:],
                        in_offset=bass.IndirectOffsetOnAxis(
                            ap=xt[:, t:t + 1], axis=0))

                    # ---------------- scaled transposes of E (g * E_k).T
                    diag = sp.tile([P, P], F32, tag="diag", name="diag")
                    nc.vector.tensor_scalar_mul(diag[:], id96[:], gt[:, t:t + 1])
                    ETs = sp.tile([D, K * P], F32, tag="ETs", bufs=3, name="ETs")
                    for k in range(K):
                        ps = pp.tile([D, P], F32, tag="mm", bufs=4,
                                     space="PSUM", name="ETk_ps")
                        nc.tensor.matmul(ps[:], lhsT=E[:, k * D:(k + 1) * D],
                                         rhs=diag[:], start=True, stop=True)
                        if k % 2 == 0:
                            nc.vector.tensor_copy(ETs[:, k * P:(k + 1) * P], ps[:])
                        else:
                            nc.scalar.copy(ETs[:, k * P:(k + 1) * P], ps[:])
                    hidT_ps = pp.tile([D, P], F32, tag="mm", bufs=4,
                                      space="PSUM", name="hidT_ps")
                    nc.tensor.transpose(hidT_ps[:], hid[:], id96[:])
                    hidT = sp.tile([D, P], F32, tag="hidT", bufs=2, name="hidT")
                    nc.vector.tensor_copy(hidT[:], hidT_ps[:])

                    # ---------------- neighbor attention logits
                    a_ps = pp.tile([P, K], F32, tag="a", bufs=1, space="PSUM",
                                   name="a_ps")
                    KC = 4  # k's per pre-chunk
                    for kc in range(K // KC):
                        pre = pp.tile([D1, KC * P], F32, tag="mm", bufs=4,
                                      space="PSUM", name="pre_ps")
                        nc.tensor.matmul(
                            pre[:], lhsT=w1d[:],
                            rhs=ETs[:, kc * KC * P:(kc + 1) * KC * P],
                            start=True, stop=False)
                        off = t * RPC * K // T + kc * KC * P
                        nc.tensor.matmul(
                            pre[:], lhsT=w1r[:],
                            rhs=wsT[:, off:off + KC * P],
                            start=False, stop=True)
                        tT = sp.tile([D1, KC * P], F32, tag="tT", bufs=3,
                                     name="tT")
                        # leaky_relu(pre + w1_b, 0.01); Lrelu table slope=.01
                        nc.scalar.activation(tT[:], pre[:], AF.Lrelu,
                                             bias=w1b[:])
                        for kk in range(KC):
                            k = kc * KC + kk
                            nc.tensor.matmul(
                                a_ps[:, k:k + 1],
                                lhsT=tT[:, kk * P:(kk + 1) * P], rhs=q1[:],
                                start=True, stop=True)

                    # ---------------- softmax over K + h_n
                    nm = sp.tile([P, 1], F32, tag="nm", name="nm")
                    nc.vector.reduce_max(nm[:], a_ps[:], axis=AX.X, negate=True)
                    ex = sp.tile([P, K], F32, tag="ex", name="ex")
                    nc.scalar.activation(ex[:], a_ps[:], AF.Exp, bias=nm[:])
                    sm = sp.tile([P, 1], F32, tag="sm", name="sm")
                    nc.vector.reduce_sum(sm[:], ex[:], axis=AX.X)
                    inv = sp.tile([P, 1], F32, tag="inv", name="inv")
                    nc.vector.reciprocal(inv[:], sm[:])
                    alp = sp.tile([P, K], F32, tag="alp", name="alp")
                    nc.vector.tensor_scalar_mul(alp[:], ex[:], inv[:])

                    Em = sp.tile([P, K * D], F32, tag="Em", bufs=2, name="Em")
                    nc.vector.tensor_tensor(
                        out=Em[:].rearrange("p (k d) -> p k d", k=K),
                        in0=E[:].rearrange("p (k d) -> p k d", k=K),
                        in1=alp[:, :, None].to_broadcast([P, K, D]),
                        op=ALU.mult)
                    hn = sp.tile([P, D], F32, tag="hn", name="hn")
                    nc.vector.reduce_sum(
                        hn[:], Em[:].rearrange("p (k d) -> p d k", k=K),
                        axis=AX.X)
                    hnT_ps = pp.tile([D, P], F32, tag="mm", bufs=4,
                                     space="PSUM", name="hnT_ps")
                    nc.tensor.transpose(hnT_ps[:], hn[:], id96[:])
                    hnT = sp.tile([D, P], F32, tag="hnT", name="hnT")
                    nc.scalar.copy(hnT[:], hnT_ps[:])

                    # ---------------- h_global = relu([hid, hn] @ w2 + b2)
                    hg_ps = pp.tile([P, D], F32, tag="mm", bufs=4,
                                    space="PSUM", name="hg_ps")
                    nc.tensor.matmul(hg_ps[:], lhsT=hidT[:], rhs=w2a[:],
                                     start=True, stop=False)
                    nc.tensor.matmul(hg_ps[:], lhsT=hnT[:], rhs=w2b[:],
                                     start=False, stop=False)
                    nc.tensor.matmul(hg_ps[:], lhsT=ones1[:], rhs=w2bias[:],
                                     start=False, stop=True)
                    hg = sp.tile([P, D], F32, tag="hg", name="hg")
                    nc.scalar.activation(hg[:], hg_ps[:], AF.Relu)

                    # ---------------- local branch
                    haT = sp.tile([D, P], F32, tag="haT", name="haT")
                    nc.vector.tensor_scalar_mul(haT[:], hidT[:], aw[:])
                    sc_ps = pp.tile([P, P], F32, tag="mm", bufs=4,
                                    space="PSUM", name="sc_ps")
                    nc.tensor.matmul(sc_ps[:], lhsT=haT[:], rhs=hidT[:],
                                     start=True, stop=True)
                    # leaky_relu(sc, 0.2) = max(sc, 0.2*sc), then mask bias
                    sc2 = sp.tile([P, P], F32, tag="sc2", name="sc2")
                    nc.scalar.mul(sc2[:], sc_ps[:], 0.2)
                    scm = sp.tile([P, P], F32, tag="scm", name="scm")
                    nc.vector.tensor_max(scm[:], sc_ps[:], sc2[:])
                    nc.vector.tensor_add(scm[:], scm[:],
                                         maskb[:, t * P:(t + 1) * P])
                    nm2 = sp.tile([P, 1], F32, tag="nm2", name="nm2")
                    nc.vector.reduce_max(nm2[:], scm[:], axis=AX.X, negate=True)
                    u = sp.tile([P, P], F32, tag="u", name="u")
                    nc.scalar.activation(u[:], scm[:], AF.Exp, bias=nm2[:])
                    sm2 = sp.tile([P, 1], F32, tag="sm2", name="sm2")
                    nc.vector.reduce_sum(sm2[:], u[:], axis=AX.X)
                    inv2 = sp.tile([P, 1], F32, tag="inv2", name="inv2")
                    nc.vector.reciprocal(inv2[:], sm2[:])
                    pnorm = sp.tile([P, P], F32, tag="pnorm", name="pnorm")
                    nc.vector.tensor_scalar_mul(pnorm[:], u[:], inv2[:])
                    pT_ps = pp.tile([P, P], F32, tag="mm", bufs=4,
                                    space="PSUM", name="pT_ps")
                    nc.tensor.transpose(pT_ps[:], pnorm[:], id96[:])
                    pT = sp.tile([P, P], F32, tag="pT", name="pT")
                    nc.scalar.copy(pT[:], pT_ps[:])
                    hl_ps = pp.tile([P, D], F32, tag="mm", bufs=4,
                                    space="PSUM", name="hl_ps")
                    nc.tensor.matmul(hl_ps[:], lhsT=pT[:], rhs=hid[:],
                                     start=True, stop=True)

                    # ---------------- h = h_local + h_global
                    h = sp.tile([P, D], F32, tag="h", bufs=2, name="h")
                    nc.vector.tensor_add(h[:], hl_ps[:], hg[:])
                    hT_ps = pp.tile([D, P], F32, tag="mm", bufs=4,
                                    space="PSUM", name="hT_ps")
                    nc.tensor.transpose(hT_ps[:], h[:], id96[:])
                    hT = sp.tile([D, P], F32, tag="hT", name="hT")
                    nc.vector.tensor_copy(hT[:], hT_ps[:])

                    # ---------------- z = tanh([h, pos] @ w3 + b3)
                    z_ps = pp.tile([P, D], F32, tag="mm", bufs=4,
                                   space="PSUM", name="z_ps")
                    nc.tensor.matmul(z_ps[:], lhsT=hT[:], rhs=w3a[:],
                                     start=True, stop=False)
                    nc.tensor.matmul(z_ps[:], lhsT=sel[:, t * P:(t + 1) * P],
                                     rhs=posw3[:], start=False, stop=False)
                    nc.tensor.matmul(z_ps[:], lhsT=ones1[:], rhs=w3bias[:],
                                     start=False, stop=True)
                    z = sp.tile([P, D], F32, tag="z", name="z")
                    nc.scalar.activation(z[:], z_ps[:], AF.Tanh)
                    zT_ps = pp.tile([D, P], F32, tag="mm", bufs=4,
                                    space="PSUM", name="zT_ps")
                    nc.tensor.transpose(zT_ps[:], z[:], id96[:])
                    zT = sp.tile([D, P], F32, tag="zT", name="zT")
                    nc.scalar.copy(zT[:], zT_ps[:])

                    # ---------------- s_ = session mean of h, then @ w5
                    s2_ps = pp.tile([2, D], F32, tag="mm", bufs=4,
                                    space="PSUM", name="s2_ps")
                    nc.tensor.matmul(s2_ps[:], lhsT=rep2T48[:], rhs=h[:],
                                     start=True, stop=True)
                    s2 = sp.tile([2, D], F32, tag="s2", name="s2")
                    nc.vector.tensor_copy(s2[:], s2_ps[:])
                    s2T_ps = pp.tile([D, 2], F32, tag="mm", bufs=4,
                                     space="PSUM", name="s2T_ps")
                    nc.tensor.transpose(s2T_ps[:], s2[:], id2[:])
                    s2T = sp.tile([D, 2], F32, tag="s2T", name="s2T")
                    nc.vector.tensor_copy(s2T[:], s2T_ps[:])
                    sw5_ps = pp.tile([2, D], F32, tag="mm", bufs=4,
                                     space="PSUM", name="sw5_ps")
                    nc.tensor.matmul(sw5_ps[:], lhsT=s2T[:], rhs=w5[:],
                                     start=True, stop=True)
                    sw5 = sp.tile([2, D], F32, tag="sw5", name="sw5")
                    nc.vector.tensor_copy(sw5[:], sw5_ps[:])

                    # ---------------- beta = sigmoid(z@w4 + s_@w5 + b5) @ q2
                    bp_ps = pp.tile([P, D], F32, tag="mm", bufs=4,
                                    space="PSUM", name="bp_ps")
                    nc.tensor.matmul(bp_ps[:], lhsT=zT[:], rhs=w4[:],
                                     start=True, stop=False)
                    nc.tensor.matmul(bp_ps[:], lhsT=rep2[:], rhs=sw5[:],
                                     start=False, stop=False)
                    nc.tensor.matmul(bp_ps[:], lhsT=ones1[:], rhs=w5bias[:],
                                     start=False, stop=True)
                    sg = sp.tile([P, D], F32, tag="sg", name="sg")
                    nc.scalar.activation(sg[:], bp_ps[:], AF.Sigmoid)
                    bq = sp.tile([P, D], F32, tag="bq", name="bq")
                    nc.vector.tensor_tensor(bq[:], sg[:], q2[:], op=ALU.mult)
                    beta = sp.tile([P, 1], F32, tag="beta", name="beta")
                    nc.vector.reduce_sum(beta[:], bq[:], axis=AX.X)

                    # ---------------- S += sum_l beta * h (per session)
                    bd = sp.tile([P, S], F32, tag="bd", name="bd")
                    nc.vector.tensor_scalar_mul(
                        bd[:], bmask[:, t * S:(t + 1) * S], beta[:])
                    nc.tensor.matmul(S_ps[:], lhsT=bd[:], rhs=h[:],
                                     start=(t == 0), stop=(t == T - 1))

                # -------------------- AllGather session vectors
                S_sb = sp.tile([S, D], F32, tag="S_sb", name="S_sb")
                nc.vector.tensor_copy(S_sb[:], S_ps[:])
                nc.sync.dma_start(s_in[:], S_sb[:])
                if fake_coll:
                    # timing-mode stand-in: collectives are illegal inside
                    # control flow; emulate the data movement locally
                    for kk in range(C):
                        nc.sync.dma_start(s_out[kk * S:(kk + 1) * S, :],
                                          s_in[:])
                else:
                    nc.gpsimd.collective_compute(
                        "AllGather", ALU.bypass,
                        replica_groups=[list(range(C))],
                        ins=[s_in.opt()], outs=[s_out.opt()])
                Sf = sp.tile([B, D], F32, tag="Sf", name="Sf")
                nc.sync.dma_start(Sf[:], s_out[:])
                ST_ps = pp.tile([D, B], F32, tag="mm", bufs=4, space="PSUM",
                                name="ST_ps")
                nc.tensor.transpose(ST_ps[:], Sf[:], id128[:])
                ST = sp.tile([D, B], F32, tag="ST", name="ST")
                nc.vector.tensor_copy(ST[:], ST_ps[:])

                # -------------------- vocab-sharded scores = S @ embT
                NV_DMA = 2048
                NV_MM = 512
                off = 0
                ci = 0
                while off < VS:
                    w = min(NV_DMA, VS - off)
                    et = sp.tile([D, NV_DMA], F32, tag="et", bufs=3, name="et")
                    nc.sync.dma_start(et[:, :w], embT_d[:, off:off + w])
                    sco = sp.tile([B, NV_DMA], F32, tag="sco", bufs=3,
                                  name="sco")
                    o2 = 0
                    while o2 < w:
                        m = min(NV_MM, w - o2)
                        sps = pp.tile([B, NV_MM], F32, tag="score", bufs=2,
                                      space="PSUM", name="sps")
                        nc.tensor.matmul(sps[:, :m], lhsT=ST[:],
                                         rhs=et[:, o2:o2 + m],
                                         start=True, stop=True)
                        if ci % 3 == 0:
                            nc.scalar.copy(sco[:, o2:o2 + m], sps[:, :m])
                        else:
                            nc.vector.tensor_copy(sco[:, o2:o2 + m],
                                                  sps[:, :m])
                        ci += 1
                        o2 += m
                    nc.scalar.dma_start(scores_d[:, off:off + w], sco[:, :w])
                    off += w

            if loop:
                with tc.For_i(0, loop, 1):
                    emit_body(0, fake_coll=True)
            else:
                for rep in range(repeat):
                    emit_body(rep)

    return nc


# ------------------------------------------------------------ host-side prep
def prep_in_maps(x, edge_index, nbr_ids, nbr_w, emb, pos_w, a_w, w1_W, w1_b,
                 q1_w, w2_W, w2_b, w3_W, w3_b, q2_w, w4_W, w5_W, w5_b):
    x = np.asarray(x)
    edge_index = np.asarray(edge_index)
    nbr_ids = np.asarray(nbr_ids)
    nbr_w = np.asarray(nbr_w, dtype=np.float32)
    emb = np.ascontiguousarray(np.asarray(emb, dtype=np.float32))

    # session mean of node ids (f32, matching jnp mean)
    s_mean = x.reshape(B, L).astype(np.float32).mean(axis=1,
                                                     dtype=np.float32)
    g_full = np.repeat(s_mean, L)                         # [N]

    # softmax of nbr_w over K (f32)
    wf = nbr_w.reshape(N, K)
    wmax = wf.max(axis=1, keepdims=True)
    we = np.exp(wf - wmax, dtype=np.float32)
    w_soft = we / we.sum(axis=1, keepdims=True, dtype=np.float32)

    # undirected within-session adjacency -> mask bias blocks
    src = np.asarray(edge_index[0])
    dst = np.asarray(edge_index[1])
    sess = src // L
    adj = np.zeros((B, L, L), dtype=bool)
    adj[sess, src % L, dst % L] = True
    m = adj | adj.transpose(0, 2, 1)
    mb = np.where(m, np.float32(0), np.float32(NEG))      # [B, L, L]

    nbr_flat = nbr_ids.reshape(N, K)

    w1_W = np.asarray(w1_W, np.float32)
    weights = {
        "w1d": np.ascontiguousarray(w1_W[:D, :]),
        "w1r": np.ascontiguousarray(w1_W[D:D + 1, :]),
        "w1b": np.ascontiguousarray(np.asarray(w1_b, np.float32)[:, None]),
        "q1": np.ascontiguousarray(np.asarray(q1_w, np.float32)[:, None]),
        "w2a": np.ascontiguousarray(np.asarray(w2_W, np.float32)[:D, :]),
        "w2b": np.ascontiguousarray(np.asarray(w2_W, np.float32)[D:, :]),
        "w2bias": np.asarray(w2_b, np.float32)[None, :],
        "w3a": np.ascontiguousarray(np.asarray(w3_W, np.float32)[:D, :]),
        "w3b": np.ascontiguousarray(np.asarray(w3_W, np.float32)[D:, :]),
        "w3bias": np.asarray(w3_b, np.float32)[None, :],
        "w4": np.asarray(w4_W, np.float32),
        "w5": np.asarray(w5_W, np.float32),
        "w5bias": np.asarray(w5_b, np.float32)[None, :],
        "q2": np.ascontiguousarray(np.broadcast_to(np.asarray(q2_w, np.float32)[None, :], (P, D))),
        "aw": np.ascontiguousarray(np.asarray(a_w, np.float32)[:, None]),
    }
    ones1 = np.ones((1, P), np.float32)
    rep2 = np.zeros((2, P), np.float32)
    rep2[0, :L] = 1.0
    rep2[1, L:] = 1.0
    rep2T48 = np.zeros((P, 2), np.float32)
    rep2T48[:L, 0] = 1.0 / L
    rep2T48[L:, 1] = 1.0 / L
    sel = np.zeros((S, T * P), np.float32)
    bmask = np.zeros((P, T * S), np.float32)
    for t in range(T):
        sel[2 * t, t * P:t * P + L] = 1.0
        sel[2 * t + 1, t * P + L:(t + 1) * P] = 1.0
        bmask[:L, t * S + 2 * t] = 1.0
        bmask[L:, t * S + 2 * t + 1] = 1.0
    weights.update({
        "ones1": ones1, "rep2": rep2, "rep2T48": rep2T48, "sel": sel,
        "bmask": bmask, "id16": np.eye(S, dtype=np.float32),
        "id2": np.eye(2, dtype=np.float32),
    })
    pos_w = np.asarray(pos_w, np.float32)

    in_maps = []
    for c in range(C):
        r0 = c * RPC
        xc = x[r0:r0 + RPC].astype(np.int32)
        xt_i = np.ascontiguousarray(xc.reshape(T, P).T)
        nbrc = nbr_flat[r0:r0 + RPC].astype(np.int32)
        nbr_i = np.ascontiguousarray(
            nbrc.reshape(T, P, K).transpose(1, 0, 2).reshape(P, T * K))
        g_i = np.ascontiguousarray(
            g_full[r0:r0 + RPC].reshape(T, P).T.astype(np.float32))
        wsc = w_soft[r0:r0 + RPC]
        wsT_i = np.ascontiguousarray(
            wsc.reshape(T, P, K).transpose(0, 2, 1).reshape(1, -1))
        blk = np.full((T, P, P), NEG, dtype=np.float32)
        for t in range(T):
            sA = c * S + 2 * t
            blk[t, 0:L, 0:L] = mb[sA]
            blk[t, L:P, L:P] = mb[sA + 1]
        maskb_i = np.ascontiguousarray(
            blk.transpose(1, 0, 2).reshape(P, T * P))
        embT_i = np.ascontiguousarray(emb[c * VS:(c + 1) * VS, :].T)
        pos_i = np.ascontiguousarray(pos_w[c * S:(c + 1) * S, :])
        im = {
            "emb": emb, "embT": embT_i, "xt": xt_i, "nbr": nbr_i,
            "g": g_i, "wsT": wsT_i, "maskb": maskb_i, "pos": pos_i,
        }
        im.update(weights)
        in_maps.append(im)
    return in_maps


def _enable_ldw_opt():
    bu = bass_utils

    def bir_verify_and_optimise(tmpdir, inp="bir.json", outp="file.neff",
                                arch=None, *, dve_root=None):
        cmd = [
            bu.get_walrus_driver(),
            "--pass",
            ",".join([
                "birverifier", "runtime_memory_reservation", "lower_act",
                "lower_dve", "lower_ap_offset", "codegen", "neff_packager",
            ]),
            "-i", inp,
            "--neff-output-filename", outp,
            "--enable-birsim=true", "--mem-mode=physical", "--policy=0",
            "--enable-ldw-opt=true",
            "--assign-static-dmas-to-sp=false",
            "--dram-page-size=256", "--enable-neff-debug-info=true",
            "--jobs", "8",
            *bu.get_walrus_args(
                bu.get_bir_arch(tmpdir, inp) if arch is None else arch,
                tmpdir, dve_root=dve_root),
        ]
        result = bu.run_command(cmd, cwd=tmpdir)
        if result is not None:
            from pathlib import Path
            (Path(tmpdir) / "log.txt").write_text(result.stdout)
        return f"{tmpdir}/{outp}"

    bu.bir_verify_and_optimise = bir_verify_and_optimise


_enable_ldw_opt()

_CACHE = {}


def kernel(**inputs) -> np.ndarray:
    in_maps = prep_in_maps(**inputs)
    if "nc" not in _CACHE:
        nc = build_nc()
        legalize_waits(nc)
        _CACHE["nc"] = nc
    nc = _CACHE["nc"]
    res = bass_utils.run_bass_kernel_spmd(
        nc, in_maps, core_ids=list(range(C))).results
    return np.concatenate([res[c]["scores"] for c in range(C)], axis=1)

